# revision 27
# baseline (speedup 1.0000x reference)
"""Multi-head self-attention (L=2048, N=4, E=1024, h=16) on 8 NeuronCores.

Sharding: core c handles batch n = c//2 and heads [8*(c%2), 8*(c%2)+8).
Each core computes q/k/v projections for its (n, head-block), attention,
and a partial out-projection (columns of out_proj for its heads).
Host sums the two partials per batch n and adds out_bias.

v4 (bf16 everywhere + fused denominator + latency-split exp):
- all matmuls bf16 (fp8 DoubleRow measured slower on HW than the cost
  model claims: the 256-col ldweights can't hide behind the shortened
  stream, and fp8 score/at/v quantization eats the 2e-2 error budget).
- k bias dropped (softmax-invariant), v bias folded into out_bias on host.
- denominator fused into attn@V: each head's v-block in vv is 66 wide
  ([0:64]=v, [64]=ones, [65]=pad), so the pv matmul's row 64 accumulates
  sum(exp(s)) -- no separate denominator matmuls (saves 512 matmuls).
- exp(scores): on most iterations the two heads run in PARALLEL on
  ScalarE (table exp) and DVE (Schraudolph bf16 bit-trick: int16 bits =
  round(s*128*log2e + 16250.5) ARE the bf16 value), halving the
  QK->exp->QK latency chain; chunk-boundary iterations run both heads on
  ScalarE while the DVE runs the normalize tail.
- normalize tail: reciprocal of the den row on partition 64, 2KB DMA hop
  to partition 0 (partition_broadcast cannot source from partition 64),
  broadcast, multiply; the j=1 half is staged and DMA'd into aoT
  partitions 64:128 (both pv psum tiles start at partition 0).
- attention processes ONE m-tile (2 heads) per chunk; per-j score tiles
  rotate through 4 PSUM banks so QK(lk+2) only waits exp(lk) of its own
  j-lane. PSUM = st 4 + pv 2 + filler/v 1.
- fillers (q/k proj of later m-tiles, out-projection of finished rows)
  are split into (matmul, psum->sbuf write) thunk pairs popped one
  iteration apart so writes never head-of-line-block the DVE queue.
"""

from contextlib import ExitStack

import ml_dtypes
import numpy as np

import concourse.bacc as bacc
import concourse.mybir as mybir
import concourse.tile as tile
from concourse.bass_utils import run_bass_kernel_spmd

L, N, E, H, D = 2048, 4, 1024, 16, 64
SCALE = D**-0.5
IL = 512  # inner dims per core (8 heads * 64)
P = 128
F32 = mybir.dt.float32
BF16 = mybir.dt.bfloat16
I16 = mybir.dt.int16
EXP = mybir.ActivationFunctionType.Exp
COPY = mybir.ActivationFunctionType.Copy
MUL = mybir.AluOpType.mult
ADD = mybir.AluOpType.add

C1 = 184.6650  # 128*log2(e)
C2 = 16250.49  # 127*128 - 5.51 (centering); HW fp32->int16 convert rounds
# On SPLIT iters j=0 -> ScalarE, j=1 -> DVE in parallel; others full-ScalarE.
SPLIT_EXP = frozenset(range(2, 13))

_built = None


def build(dbg=False, reps=1, loop_reps=1):
    nc = bacc.Bacc("TRN2", target_bir_lowering=False, debug=False, num_devices=8)

    # [128, 8, F] layouts: E dim = 128*t + p
    qt_d = nc.dram_tensor("qt", [P, 8, L], BF16, kind="ExternalInput")
    w_d = nc.dram_tensor("w", [P, 8, 3, IL], BF16, kind="ExternalInput")
    bq_d = nc.dram_tensor("bq", [4, P], F32, kind="ExternalInput")
    opt_d = nc.dram_tensor("opt", [IL, E], BF16, kind="ExternalInput")
    out_d = nc.dram_tensor("out", [L, E], F32, kind="ExternalOutput")

    with tile.TileContext(nc) as tc:
      lctx = tc.For_i(0, loop_reps, 1) if loop_reps > 1 else None
      if lctx is not None:
          lctx.__enter__()
      for _rep in range(reps):
        est = ExitStack()
        persist = est.enter_context(tc.tile_pool(name="persist", bufs=1))

        bq_sb = persist.tile([P, 4], F32, name="bq_sb")
        for m in range(4):
            nc.sync.dma_start(out=bq_sb[:, m : m + 1], in_=bq_d[m, :, None])

        qT = [persist.tile([P, L], BF16, name=f"qT{m}") for m in range(4)]
        kT = [persist.tile([P, L], BF16, name=f"kT{m}") for m in range(4)]
        # v padded per head to 66 cols: [0:64]=v dims, [64]=ones (denominator
        # row of the pv matmul), [65]=pad. Layout [part(k), lk, head, col].
        vv = persist.tile([P, 16, 8, 66], BF16, name="vv")
        nc.vector.memset(vv[:, :, :, 64:66], 0.0)
        nc.vector.memset(vv[:, :, :, 64:65], 1.0)
        aoT = [persist.tile([P, L], BF16, name=f"aoT{m}") for m in range(4)]
        opt_sb = [persist.tile([P, E], BF16, name=f"opt{k}") for k in range(4)]
        for k in range(4):
            nc.sync.dma_start(out=opt_sb[k], in_=opt_d[k * P : (k + 1) * P, :])

        # ---------------- phase 1 setup: streaming inputs ----------------
        ph_all = est.enter_context(ExitStack())
        in_pool = ph_all.enter_context(tc.tile_pool(name="in_pool", bufs=1))
        qt_sb = in_pool.tile([P, 8, L], BF16, tag="qt", name="qtsb")
        w_sb = in_pool.tile([P, 8, 3, IL], BF16, tag="w", name="wsb")
        for s in range(8):
            nc.sync.dma_start(out=qt_sb[:, s, :], in_=qt_d[:, s, :])
            nc.sync.dma_start(out=w_sb[:, s, 0:2, :], in_=w_d[:, s, 0:2, :])
        for s in range(8):
            nc.sync.dma_start(out=w_sb[:, s, 2, :], in_=w_d[:, s, 2, :])

        def qk_write(ps, m, ch, nm):
            chs = slice(ch * 512, (ch + 1) * 512)
            if nm == "q":
                nc.vector.tensor_scalar(
                    out=qT[m][:, chs],
                    in0=ps,
                    scalar1=SCALE,
                    scalar2=bq_sb[:, m : m + 1],
                    op0=MUL,
                    op1=ADD,
                )
            else:
                nc.vector.tensor_copy(out=kT[m][:, chs], in_=ps)

        def qk_mms(ps_pool, m, ch, col, nm):
            chs = slice(ch * 512, (ch + 1) * 512)
            mslc = slice(m * P, (m + 1) * P)
            ps = ps_pool.tile([P, 512], F32, tag="qkps", name=f"ps{nm}{m}{ch}")
            for s in range(8):
                nc.tensor.matmul(
                    ps,
                    w_sb[:, s, col, mslc],
                    qt_sb[:, s, chs],
                    start=(s == 0),
                    stop=(s == 7),
                )
            return ps

        # q/k projection Mtiles 0..1 up front (then pool closed)
        with tc.tile_pool(name="qk_ps", bufs=2, space="PSUM") as qk_ps:
            for m in (0, 1):
                for ch in range(4):
                    for col, nm in ((0, "q"), (1, "k")):
                        ps = qk_mms(qk_ps, m, ch, col, nm)
                        qk_write(ps, m, ch, nm)

        # ---------------- phase 2: attention with interleaved fillers ----------
        with ExitStack() as ph2:
            at_pool = ph2.enter_context(tc.tile_pool(name="at", bufs=4))
            small = ph2.enter_context(tc.tile_pool(name="small", bufs=4))
            osb = ph2.enter_context(tc.tile_pool(name="osb", bufs=3))
            st_ps = ph2.enter_context(tc.tile_pool(name="st", bufs=4, space="PSUM"))
            pv_ps = ph2.enter_context(tc.tile_pool(name="pv", bufs=1, space="PSUM"))

            fillers = []  # deque of thunks, each ~0.5-2us of PE work
            exp_ctr = [0]

            # Filler thunks come in (matmul, write) pairs popped one iteration
            # apart so the PSUM->SBUF write never head-of-line-blocks the DVE
            # queue while its own matmuls are still pending.
            def make_qk23_fillers(ps_pool):
                for m in (2, 3):
                    for ch in range(4):
                        for col, nm in ((0, "q"), (1, "k")):
                            cell = []

                            def t_mm(cell=cell, m=m, ch=ch, col=col, nm=nm):
                                cell.append(qk_mms(ps_pool, m, ch, col, nm))

                            def t_wr(cell=cell, m=m, ch=ch, nm=nm):
                                qk_write(cell.pop(), m, ch, nm)

                            fillers.append(t_mm)
                            fillers.append(t_wr)

            def make_outproj_fillers(ps_pool, lts):
                for lt in lts:
                    for c in (0, 1):
                        cell = []

                        def t_mm(cell=cell, lt=lt, c=c):
                            ps = ps_pool.tile(
                                [P, 512], F32, tag="ops", name=f"ops{lt}{c}"
                            )
                            for k in range(4):
                                nc.tensor.matmul(
                                    ps,
                                    aoT[k][:, lt * P : (lt + 1) * P],
                                    opt_sb[k][:, c * 512 : (c + 1) * 512],
                                    start=(k == 0),
                                    stop=(k == 3),
                                )
                            cell.append(ps)

                        def t_wr(cell=cell, lt=lt, c=c):
                            ob = osb.tile([P, 512], F32, tag="ob", name=f"ob{lt}{c}")
                            nc.vector.tensor_copy(out=ob, in_=cell.pop())
                            nc.sync.dma_start(
                                out=out_d[lt * P : (lt + 1) * P, c * 512 : (c + 1) * 512],
                                in_=ob,
                            )

                        fillers.append(t_mm)
                        fillers.append(t_wr)

            def attn_chunk(m, lq, v_interleave, fill_budget):
                """One m-tile (2 heads), one 512-wide q chunk."""
                lqs = slice(lq * 512, (lq + 1) * 512)
                # pv rows: [0:64]=attn@V dims, [64]=denominator
                pv_t = [
                    pv_ps.tile([65, 512], F32, tag=f"pv{j}", name=f"pv_{m}_{lq}_{j}")
                    for j in (0, 1)
                ]

                def pv_step(lk, at):
                    for j in (0, 1):
                        nc.tensor.matmul(
                            pv_t[j],
                            vv[:, lk, 2 * m + j, 0:65],
                            at[:, j, :],
                            start=(lk == 0),
                            stop=(lk == 15),
                        )

                prev = None  # (lk, at tile)
                for lk in range(16):
                    lks = slice(lk * P, (lk + 1) * P)
                    cur = at_pool.tile([P, 2, 512], BF16, tag="at", name=f"at_{m}_{lq}_{lk}")
                    # per-j score tiles, 4-deep rotation: QK(lk+2, j) only
                    # waits on exp(lk, j) of its own j-lane
                    st = [
                        st_ps.tile([P, 512], F32, tag="st", name=f"st_{m}_{lq}_{lk}_{j}")
                        for j in (0, 1)
                    ]
                    for j in (0, 1):
                        nc.tensor.matmul(
                            st[j],
                            kT[m][64 * j : 64 * j + 64, lks],
                            qT[m][64 * j : 64 * j + 64, lqs],
                            start=True,
                            stop=True,
                        )
                    c = exp_ctr[0]
                    exp_ctr[0] += 1
                    nc.scalar.activation(out=cur[:, 0, :], in_=st[0], func=EXP)
                    if (c % 16) in SPLIT_EXP:
                        nc.vector.tensor_scalar(
                            out=cur[:, 1, :].bitcast(I16),
                            in0=st[1],
                            scalar1=C1,
                            scalar2=C2,
                            op0=MUL,
                            op1=ADD,
                        )
                    else:
                        nc.scalar.activation(out=cur[:, 1, :], in_=st[1], func=EXP)
                    if prev is not None:
                        pv_step(prev[0], prev[1])
                    prev = (lk, cur)
                    if v_interleave is not None:
                        v_interleave(lk)
                    for _ in range(fill_budget):
                        if fillers:
                            fillers.pop(0)()
                pv_step(prev[0], prev[1])

                # tail: recip@p64 -> DMA hop to p0 -> broadcast -> normalize
                bcs = small.tile([P, 2, 512], F32, tag="bcs", name=f"bcs_{m}_{lq}", bufs=2)
                rc = small.tile([65, 2, 512], F32, tag="rc", name=f"rc_{m}_{lq}", bufs=2)
                rc0 = small.tile([1, 2, 512], F32, tag="rc0", name=f"rc0_{m}_{lq}", bufs=2)
                stage = osb.tile([64, 512], BF16, tag="aostg", name=f"stg_{m}_{lq}")
                for j in (0, 1):
                    nc.vector.reciprocal(
                        out=rc[64:65, j, :], in_=pv_t[j][64:65, :]
                    )
                    nc.sync.dma_start(out=rc0[:, j, :], in_=rc[64:65, j, :])
                    nc.gpsimd.partition_broadcast(bcs[:, j, :], rc0[:, j, :])
                for j in (0, 1):
                    nc.vector.tensor_mul(
                        out=aoT[m][0:64, lqs] if j == 0 else stage,
                        in0=pv_t[j][0:64, :],
                        in1=bcs[0:64, j, :],
                    )
                nc.sync.dma_start(out=aoT[m][64:128, lqs], in_=stage)

            # (lq0, m0): v projection rides inside the lk loop
            with tc.tile_pool(name="v_ps", bufs=1, space="PSUM") as v_ps:
                def v_interleave(lk):
                    ps = v_ps.tile([P, 8, 64], F32, tag="vps", name=f"psv{lk}")
                    lkc = slice(lk * P, (lk + 1) * P)
                    for s in range(8):
                        nc.tensor.matmul(
                            ps,
                            qt_sb[:, s, lkc],
                            w_sb[:, s, 2, :],
                            start=(s == 0),
                            stop=(s == 7),
                        )
                    nc.scalar.activation(out=vv[:, lk, :, 0:64], in_=ps, func=COPY)

                attn_chunk(0, 0, v_interleave, 0)

            # rest of lq0: q/k Mtiles 2,3 fill PE idle
            with tc.tile_pool(name="qk2_ps", bufs=1, space="PSUM") as qk2_ps:
                make_qk23_fillers(qk2_ps)
                for m in (1, 2, 3):
                    attn_chunk(m, 0, None, 1)
                while fillers:
                    fillers.pop(0)()

            # lq 1-3: out-projection of previous lq's rows fills PE idle
            with tc.tile_pool(name="o_ps", bufs=1, space="PSUM") as o_ps:
                for lq in range(1, 4):
                    make_outproj_fillers(o_ps, range(4 * (lq - 1), 4 * lq))
                    for m in range(4):
                        attn_chunk(m, lq, None, 2)
                make_outproj_fillers(o_ps, range(12, 16))
                while fillers:
                    fillers.pop(0)()

        est.close()

      if lctx is not None:
          lctx.__exit__(None, None, None)

    nc.compile()
    return nc


def _to_pairs(a):
    """[E, F] -> [128, 8, F] with E dim = 128*t + p."""
    Edim, F = a.shape
    return np.ascontiguousarray(a.reshape(8, P, F).transpose(1, 0, 2))


def _prep_inputs(query, qkv_proj, qkv_bias, out_proj):
    """Per-core input shards (host-side)."""
    bft = ml_dtypes.bfloat16
    query = np.asarray(query, dtype=np.float32)
    qkv_proj = np.asarray(qkv_proj, dtype=np.float32)
    qkv_bias = np.asarray(qkv_bias, dtype=np.float32)
    W3 = qkv_proj.reshape(E, 3, E)  # [i, c, e], row f = 3*i + c
    b3 = qkv_bias.reshape(E, 3)
    maps = []
    for c in range(8):
        n, half = c // 2, c % 2
        isl = slice(IL * half, IL * half + IL)
        w = np.stack(
            [
                _to_pairs(W3[isl, 0, :].T),
                _to_pairs(W3[isl, 1, :].T),
                _to_pairs(W3[isl, 2, :].T),
            ],
            axis=2,
        )
        maps.append(
            {
                "qt": _to_pairs(np.ascontiguousarray(query[:, n, :].T)).astype(bft),
                "w": w.astype(bft),
                "bq": np.ascontiguousarray((b3[isl, 0] * SCALE).reshape(4, P)),
                "opt": np.ascontiguousarray(out_proj[:, isl].T).astype(bft),
            }
        )
    return maps


def kernel(query, qkv_proj, qkv_bias, out_proj, out_bias, **run_kwargs):
    global _built
    out_proj = np.asarray(out_proj, dtype=np.float32)
    out_bias = np.asarray(out_bias, dtype=np.float32)
    qkv_bias_f = np.asarray(qkv_bias, dtype=np.float32)
    bv = qkv_bias_f.reshape(E, 3)[:, 2]
    out_bias_eff = out_bias + out_proj @ bv  # v-bias folded (attn rows sum to 1)
    if _built is None:
        _built = build()
    in_maps = _prep_inputs(query, qkv_proj, qkv_bias, out_proj)
    res = run_bass_kernel_spmd(_built, in_maps, core_ids=list(range(8)), **run_kwargs)
    parts = [r["out"] for r in res.results]
    out = np.empty((L, N, E), dtype=np.float32)
    for n in range(N):
        out[:, n, :] = parts[2 * n] + parts[2 * n + 1] + out_bias_eff
    kernel.last_result = res
    return out


# revision 28
# speedup vs baseline: 1.2388x; 1.2388x over previous
"""Multi-head self-attention (L=2048, N=4, E=1024, h=16) on 8 NeuronCores.

Sharding: core c handles batch n = c//2 and heads [8*(c%2), 8*(c%2)+8).
Each core computes q/k/v projections for its (n, head-block), attention,
and a partial out-projection (columns of out_proj for its heads).
Host sums the two partials per batch n and adds out_bias.

v4 (bf16 everywhere + fused denominator + latency-split exp):
- all matmuls bf16 (fp8 DoubleRow measured slower on HW than the cost
  model claims: the 256-col ldweights can't hide behind the shortened
  stream, and fp8 score/at/v quantization eats the 2e-2 error budget).
- k bias dropped (softmax-invariant), v bias folded into out_bias on host.
- denominator fused into attn@V: each head's v-block in vv is 66 wide
  ([0:64]=v, [64]=ones, [65]=pad), so the pv matmul's row 64 accumulates
  sum(exp(s)) -- no separate denominator matmuls (saves 512 matmuls).
- exp(scores): on most iterations the two heads run in PARALLEL on
  ScalarE (table exp) and DVE (Schraudolph bf16 bit-trick: int16 bits =
  round(s*128*log2e + 16250.5) ARE the bf16 value), halving the
  QK->exp->QK latency chain; chunk-boundary iterations run both heads on
  ScalarE while the DVE runs the normalize tail.
- normalize tail: reciprocal of the den row on partition 64, 2KB DMA hop
  to partition 0 (partition_broadcast cannot source from partition 64),
  broadcast, multiply; the j=1 half is staged and DMA'd into aoT
  partitions 64:128 (both pv psum tiles start at partition 0).
- attention processes ONE m-tile (2 heads) per chunk; per-j score tiles
  rotate through 4 PSUM banks so QK(lk+2) only waits exp(lk) of its own
  j-lane. PSUM = st 4 + pv 2 + filler/v 1.
- fillers (q/k proj of later m-tiles, out-projection of finished rows)
  are split into (matmul, psum->sbuf write) thunk pairs popped one
  iteration apart so writes never head-of-line-block the DVE queue.
"""

from contextlib import ExitStack

import ml_dtypes
import numpy as np

import concourse.bacc as bacc
import concourse.mybir as mybir
import concourse.tile as tile
from concourse.bass_utils import run_bass_kernel_spmd

L, N, E, H, D = 2048, 4, 1024, 16, 64
SCALE = D**-0.5
IL = 512  # inner dims per core (8 heads * 64)
P = 128
F32 = mybir.dt.float32
BF16 = mybir.dt.bfloat16
I16 = mybir.dt.int16
EXP = mybir.ActivationFunctionType.Exp
COPY = mybir.ActivationFunctionType.Copy
MUL = mybir.AluOpType.mult
ADD = mybir.AluOpType.add

C1 = 184.6650  # 128*log2(e)
C2 = 16250.49  # 127*128 - 5.51 (centering); HW fp32->int16 convert rounds
# On SPLIT iters j=0 -> ScalarE, j=1 -> DVE in parallel; others full-ScalarE.
SPLIT_EXP = frozenset(range(2, 13))

_built = None


def build(dbg=False, reps=1, loop_reps=1):
    nc = bacc.Bacc("TRN2", target_bir_lowering=False, debug=False, num_devices=8)

    # [128, 8, F] layouts: E dim = 128*t + p
    qt_d = nc.dram_tensor("qt", [P, 8, L], BF16, kind="ExternalInput")
    w_d = nc.dram_tensor("w", [P, 8, 3, IL], BF16, kind="ExternalInput")
    bq_d = nc.dram_tensor("bq", [4, P], F32, kind="ExternalInput")
    opt_d = nc.dram_tensor("opt", [IL, E], BF16, kind="ExternalInput")
    out_d = nc.dram_tensor("out", [L, E], F32, kind="ExternalOutput")

    with tile.TileContext(nc) as tc:
      lctx = tc.For_i(0, loop_reps, 1) if loop_reps > 1 else None
      if lctx is not None:
          lctx.__enter__()
      for _rep in range(reps):
        est = ExitStack()
        persist = est.enter_context(tc.tile_pool(name="persist", bufs=1))

        bq_sb = persist.tile([P, 4], F32, name="bq_sb")
        for m in range(4):
            nc.sync.dma_start(out=bq_sb[:, m : m + 1], in_=bq_d[m, :, None])

        qT = [persist.tile([P, L], BF16, name=f"qT{m}") for m in range(4)]
        kT = [persist.tile([P, L], BF16, name=f"kT{m}") for m in range(4)]
        # v padded per head to 66 cols: [0:64]=v dims, [64]=ones (denominator
        # row of the pv matmul), [65]=pad. Layout [part(k), lk, head, col].
        vv = persist.tile([P, 16, 8, 66], BF16, name="vv")
        nc.vector.memset(vv[:, :, :, 64:66], 0.0)
        nc.vector.memset(vv[:, :, :, 64:65], 1.0)
        aoT = [persist.tile([P, L], BF16, name=f"aoT{m}") for m in range(4)]
        opt_sb = [persist.tile([P, E], BF16, name=f"opt{k}") for k in range(4)]
        for k in range(4):
            nc.sync.dma_start(out=opt_sb[k], in_=opt_d[k * P : (k + 1) * P, :])

        # ---------------- phase 1 setup: streaming inputs ----------------
        ph_all = est.enter_context(ExitStack())
        in_pool = ph_all.enter_context(tc.tile_pool(name="in_pool", bufs=1))
        qt_sb = in_pool.tile([P, 8, L], BF16, tag="qt", name="qtsb")
        w_sb = in_pool.tile([P, 8, 3, IL], BF16, tag="w", name="wsb")
        for s in range(8):
            nc.sync.dma_start(out=qt_sb[:, s, :], in_=qt_d[:, s, :])
            nc.sync.dma_start(out=w_sb[:, s, 0:2, :], in_=w_d[:, s, 0:2, :])
        for s in range(8):
            nc.sync.dma_start(out=w_sb[:, s, 2, :], in_=w_d[:, s, 2, :])

        def qk_write(ps, m, ch, nm):
            chs = slice(ch * 512, (ch + 1) * 512)
            if nm == "q":
                nc.vector.tensor_scalar(
                    out=qT[m][:, chs],
                    in0=ps,
                    scalar1=SCALE,
                    scalar2=bq_sb[:, m : m + 1],
                    op0=MUL,
                    op1=ADD,
                )
            else:
                nc.vector.tensor_copy(out=kT[m][:, chs], in_=ps)

        def qk_mms(ps_pool, m, ch, col, nm):
            chs = slice(ch * 512, (ch + 1) * 512)
            mslc = slice(m * P, (m + 1) * P)
            ps = ps_pool.tile([P, 512], F32, tag="qkps", name=f"ps{nm}{m}{ch}")
            for s in range(8):
                nc.tensor.matmul(
                    ps,
                    w_sb[:, s, col, mslc],
                    qt_sb[:, s, chs],
                    start=(s == 0),
                    stop=(s == 7),
                )
            return ps

        # q/k projection Mtiles 0..1 up front (then pool closed)
        with tc.tile_pool(name="qk_ps", bufs=2, space="PSUM") as qk_ps:
            for m in (0, 1):
                for ch in range(4):
                    for col, nm in ((0, "q"), (1, "k")):
                        ps = qk_mms(qk_ps, m, ch, col, nm)
                        qk_write(ps, m, ch, nm)

        # ---------------- phase 2: attention with interleaved fillers ----------
        with ExitStack() as ph2:
            at_pool = ph2.enter_context(tc.tile_pool(name="at", bufs=4))
            small = ph2.enter_context(tc.tile_pool(name="small", bufs=4))
            osb = ph2.enter_context(tc.tile_pool(name="osb", bufs=3))
            st_ps = ph2.enter_context(tc.tile_pool(name="st", bufs=4, space="PSUM"))
            pv_ps = ph2.enter_context(tc.tile_pool(name="pv", bufs=1, space="PSUM"))

            fillers = []  # deque of thunks, each ~0.5-2us of PE work
            exp_ctr = [0]

            # Filler thunks come in (matmul, write) pairs popped one iteration
            # apart so the PSUM->SBUF write never head-of-line-blocks the DVE
            # queue while its own matmuls are still pending.
            def make_qk23_fillers(ps_pool):
                for m in (2, 3):
                    for ch in range(4):
                        for col, nm in ((0, "q"), (1, "k")):
                            cell = []

                            def t_mm(cell=cell, m=m, ch=ch, col=col, nm=nm):
                                cell.append(qk_mms(ps_pool, m, ch, col, nm))

                            def t_wr(cell=cell, m=m, ch=ch, nm=nm):
                                qk_write(cell.pop(), m, ch, nm)

                            fillers.append(t_mm)
                            fillers.append(t_wr)

            def make_outproj_fillers(ps_pool, lts):
                for lt in lts:
                    for c in (0, 1):
                        cell = []

                        def t_mm(cell=cell, lt=lt, c=c):
                            ps = ps_pool.tile(
                                [P, 512], F32, tag="ops", name=f"ops{lt}{c}"
                            )
                            for k in range(4):
                                nc.tensor.matmul(
                                    ps,
                                    aoT[k][:, lt * P : (lt + 1) * P],
                                    opt_sb[k][:, c * 512 : (c + 1) * 512],
                                    start=(k == 0),
                                    stop=(k == 3),
                                )
                            cell.append(ps)

                        def t_wr(cell=cell, lt=lt, c=c):
                            ob = osb.tile([P, 512], F32, tag="ob", name=f"ob{lt}{c}")
                            nc.vector.tensor_copy(out=ob, in_=cell.pop())
                            nc.sync.dma_start(
                                out=out_d[lt * P : (lt + 1) * P, c * 512 : (c + 1) * 512],
                                in_=ob,
                            )

                        fillers.append(t_mm)
                        fillers.append(t_wr)

            def attn_chunk(m, lq, v_interleave, fill_budget):
                """One m-tile (2 heads), one 512-wide q chunk."""
                lqs = slice(lq * 512, (lq + 1) * 512)
                # pv rows: [0:64]=attn@V dims, [64]=denominator
                pv_t = [
                    pv_ps.tile([65, 512], F32, tag=f"pv{j}", name=f"pv_{m}_{lq}_{j}")
                    for j in (0, 1)
                ]

                def pv_step(lk, at):
                    for j in (0, 1):
                        nc.tensor.matmul(
                            pv_t[j],
                            vv[:, lk, 2 * m + j, 0:65],
                            at[:, j, :],
                            start=(lk == 0),
                            stop=(lk == 15),
                        )

                prev = None  # (lk, at tile)
                for lk in range(16):
                    lks = slice(lk * P, (lk + 1) * P)
                    cur = at_pool.tile([P, 2, 512], BF16, tag="at", name=f"at_{m}_{lq}_{lk}")
                    # per-j score tiles, 4-deep rotation: QK(lk+2, j) only
                    # waits on exp(lk, j) of its own j-lane
                    st = [
                        st_ps.tile([P, 512], F32, tag="st", name=f"st_{m}_{lq}_{lk}_{j}")
                        for j in (0, 1)
                    ]
                    for j in (0, 1):
                        nc.tensor.matmul(
                            st[j],
                            kT[m][64 * j : 64 * j + 64, lks],
                            qT[m][64 * j : 64 * j + 64, lqs],
                            start=True,
                            stop=True,
                        )
                    c = exp_ctr[0]
                    exp_ctr[0] += 1
                    nc.scalar.activation(out=cur[:, 0, :], in_=st[0], func=EXP)
                    if (c % 16) in SPLIT_EXP:
                        nc.vector.tensor_scalar(
                            out=cur[:, 1, :].bitcast(I16),
                            in0=st[1],
                            scalar1=C1,
                            scalar2=C2,
                            op0=MUL,
                            op1=ADD,
                        )
                    else:
                        nc.scalar.activation(out=cur[:, 1, :], in_=st[1], func=EXP)
                    if prev is not None:
                        pv_step(prev[0], prev[1])
                    prev = (lk, cur)
                    if v_interleave is not None:
                        v_interleave(lk)
                    for _ in range(fill_budget):
                        if fillers:
                            fillers.pop(0)()
                pv_step(prev[0], prev[1])

                # Copy pv PSUM->SBUF first: frees the pv banks so the next
                # chunk's accumulation isn't gated on the slow normalize tail
                # (reciprocal -> DMA hop -> Q7 broadcast have multi-us HW
                # latency); the tail then runs entirely from SBUF, overlapped.
                pvs = small.tile([65, 2, 512], F32, tag="pvs", name=f"pvs_{m}_{lq}", bufs=2)
                for j in (0, 1):
                    nc.vector.tensor_copy(out=pvs[:, j, :], in_=pv_t[j])
                bcs = small.tile([P, 2, 512], F32, tag="bcs", name=f"bcs_{m}_{lq}", bufs=2)
                rc = small.tile([65, 2, 512], F32, tag="rc", name=f"rc_{m}_{lq}", bufs=2)
                rc0 = small.tile([1, 2, 512], F32, tag="rc0", name=f"rc0_{m}_{lq}", bufs=2)
                stage = osb.tile([64, 512], BF16, tag="aostg", name=f"stg_{m}_{lq}")
                for j in (0, 1):
                    nc.vector.reciprocal(
                        out=rc[64:65, j, :], in_=pvs[64:65, j, :]
                    )
                    nc.sync.dma_start(out=rc0[:, j, :], in_=rc[64:65, j, :])
                    nc.gpsimd.partition_broadcast(bcs[:, j, :], rc0[:, j, :])
                for j in (0, 1):
                    nc.vector.tensor_mul(
                        out=aoT[m][0:64, lqs] if j == 0 else stage,
                        in0=pvs[0:64, j, :],
                        in1=bcs[0:64, j, :],
                    )
                nc.sync.dma_start(out=aoT[m][64:128, lqs], in_=stage)

            # (lq0, m0): v projection rides inside the lk loop
            with tc.tile_pool(name="v_ps", bufs=1, space="PSUM") as v_ps:
                def v_interleave(lk):
                    ps = v_ps.tile([P, 8, 64], F32, tag="vps", name=f"psv{lk}")
                    lkc = slice(lk * P, (lk + 1) * P)
                    for s in range(8):
                        nc.tensor.matmul(
                            ps,
                            qt_sb[:, s, lkc],
                            w_sb[:, s, 2, :],
                            start=(s == 0),
                            stop=(s == 7),
                        )
                    nc.scalar.activation(out=vv[:, lk, :, 0:64], in_=ps, func=COPY)

                attn_chunk(0, 0, v_interleave, 0)

            # rest of lq0: q/k Mtiles 2,3 fill PE idle
            with tc.tile_pool(name="qk2_ps", bufs=1, space="PSUM") as qk2_ps:
                make_qk23_fillers(qk2_ps)
                for m in (1, 2, 3):
                    attn_chunk(m, 0, None, 1)
                while fillers:
                    fillers.pop(0)()

            # lq 1-3: out-projection of previous lq's rows fills PE idle
            with tc.tile_pool(name="o_ps", bufs=1, space="PSUM") as o_ps:
                for lq in range(1, 4):
                    make_outproj_fillers(o_ps, range(4 * (lq - 1), 4 * lq))
                    for m in range(4):
                        attn_chunk(m, lq, None, 2)
                make_outproj_fillers(o_ps, range(12, 16))
                while fillers:
                    fillers.pop(0)()

        est.close()

      if lctx is not None:
          lctx.__exit__(None, None, None)

    nc.compile()
    return nc


def _to_pairs(a):
    """[E, F] -> [128, 8, F] with E dim = 128*t + p."""
    Edim, F = a.shape
    return np.ascontiguousarray(a.reshape(8, P, F).transpose(1, 0, 2))


def _prep_inputs(query, qkv_proj, qkv_bias, out_proj):
    """Per-core input shards (host-side)."""
    bft = ml_dtypes.bfloat16
    query = np.asarray(query, dtype=np.float32)
    qkv_proj = np.asarray(qkv_proj, dtype=np.float32)
    qkv_bias = np.asarray(qkv_bias, dtype=np.float32)
    W3 = qkv_proj.reshape(E, 3, E)  # [i, c, e], row f = 3*i + c
    b3 = qkv_bias.reshape(E, 3)
    maps = []
    for c in range(8):
        n, half = c // 2, c % 2
        isl = slice(IL * half, IL * half + IL)
        w = np.stack(
            [
                _to_pairs(W3[isl, 0, :].T),
                _to_pairs(W3[isl, 1, :].T),
                _to_pairs(W3[isl, 2, :].T),
            ],
            axis=2,
        )
        maps.append(
            {
                "qt": _to_pairs(np.ascontiguousarray(query[:, n, :].T)).astype(bft),
                "w": w.astype(bft),
                "bq": np.ascontiguousarray((b3[isl, 0] * SCALE).reshape(4, P)),
                "opt": np.ascontiguousarray(out_proj[:, isl].T).astype(bft),
            }
        )
    return maps


def kernel(query, qkv_proj, qkv_bias, out_proj, out_bias, **run_kwargs):
    global _built
    out_proj = np.asarray(out_proj, dtype=np.float32)
    out_bias = np.asarray(out_bias, dtype=np.float32)
    qkv_bias_f = np.asarray(qkv_bias, dtype=np.float32)
    bv = qkv_bias_f.reshape(E, 3)[:, 2]
    out_bias_eff = out_bias + out_proj @ bv  # v-bias folded (attn rows sum to 1)
    if _built is None:
        _built = build()
    in_maps = _prep_inputs(query, qkv_proj, qkv_bias, out_proj)
    res = run_bass_kernel_spmd(_built, in_maps, core_ids=list(range(8)), **run_kwargs)
    parts = [r["out"] for r in res.results]
    out = np.empty((L, N, E), dtype=np.float32)
    for n in range(N):
        out[:, n, :] = parts[2 * n] + parts[2 * n + 1] + out_bias_eff
    kernel.last_result = res
    return out


# revision 29
# speedup vs baseline: 1.3953x; 1.1264x over previous
"""Multi-head self-attention (L=2048, N=4, E=1024, h=16) on 8 NeuronCores.

Sharding: core c handles batch n = c//2 and heads [8*(c%2), 8*(c%2)+8).
Each core computes q/k/v projections for its (n, head-block), attention,
and a partial out-projection (columns of out_proj for its heads).
Host sums the two partials per batch n and adds out_bias.

PE strategy (all operands bf16, accumulation fp32 in PSUM):
- q/k/v projections: K=128 matmuls over 8 E-tiles.
- QK^T: row-packed pairs (two K=64 matmuls on row groups 0-1/2-3 run
  concurrently in the PE array).
- softmax: no max-subtraction (scores are small by construction);
  denominators via M=1 ones-matmuls, 4 heads col-packed per 32-strips;
  reciprocal on DVE, broadcast via gpsimd partition_broadcast.
- attn @ V: col-packed pairs (M=64 at tile_position (0,0)/(0,64)).
- out projection: K=128 over 4 stacked head-pair tiles.
"""

from contextlib import ExitStack

import ml_dtypes
import numpy as np

import concourse.bacc as bacc
import concourse.mybir as mybir
import concourse.tile as tile
from concourse.bass_utils import run_bass_kernel_spmd

L, N, E, H, D = 2048, 4, 1024, 16, 64
SCALE = D**-0.5
IL = 512  # inner dims per core (8 heads * 64)
P = 128
F32 = mybir.dt.float32
BF16 = mybir.dt.bfloat16
EXP = mybir.ActivationFunctionType.Exp

_built = None


def build(dbg=False, reps=1, loop_reps=1):
    nc = bacc.Bacc("TRN2", target_bir_lowering=False, debug=False, num_devices=8)

    qt_d = nc.dram_tensor("qt", [E, L], BF16, kind="ExternalInput")
    wq_d = nc.dram_tensor("wq", [E, IL], BF16, kind="ExternalInput")
    wk_d = nc.dram_tensor("wk", [E, IL], BF16, kind="ExternalInput")
    wv_d = nc.dram_tensor("wv", [E, IL], BF16, kind="ExternalInput")
    bq_d = nc.dram_tensor("bq", [4, P], F32, kind="ExternalInput")
    bk_d = nc.dram_tensor("bk", [4, P], F32, kind="ExternalInput")
    bvb_d = nc.dram_tensor("bvb", [P, IL], F32, kind="ExternalInput")
    opt_d = nc.dram_tensor("opt", [IL, E], BF16, kind="ExternalInput")
    out_d = nc.dram_tensor("out", [L, E], F32, kind="ExternalOutput")

    with tile.TileContext(nc) as tc:
      lctx = tc.For_i(0, loop_reps, 1) if loop_reps > 1 else None
      if lctx is not None:
          lctx.__enter__()
      for _rep in range(reps):
        est = ExitStack()
        persist = est.enter_context(tc.tile_pool(name="persist", bufs=1))

        ones_col = persist.tile([P, 1], BF16, name="ones_col")
        nc.vector.memset(ones_col, 1.0)

        bq_sb = persist.tile([P, 4], F32, name="bq_sb")
        bk_sb = persist.tile([P, 4], F32, name="bk_sb")
        for m in range(4):
            nc.sync.dma_start(out=bq_sb[:, m : m + 1], in_=bq_d[m, :, None])
            nc.sync.dma_start(out=bk_sb[:, m : m + 1], in_=bk_d[m, :, None])
        bvb_sb = persist.tile([P, IL], F32, name="bvb_sb")
        nc.sync.dma_start(out=bvb_sb, in_=bvb_d[:, :])

        qT = [persist.tile([P, L], BF16, name=f"qT{m}") for m in range(4)]
        kT = [persist.tile([P, L], BF16, name=f"kT{m}") for m in range(4)]
        vv = [persist.tile([P, IL], BF16, name=f"v{t}") for t in range(16)]
        aoT = [persist.tile([P, L], BF16, name=f"aoT{m}") for m in range(4)]
        opt_sb = [persist.tile([P, E], BF16, name=f"opt{k}") for k in range(4)]
        for k in range(4):
            nc.sync.dma_start(out=opt_sb[k], in_=opt_d[k * P : (k + 1) * P, :])

        # ---------------- phase 1 setup: streaming inputs ----------------
        ph_all = est.enter_context(ExitStack())
        qt_pool = ph_all.enter_context(tc.tile_pool(name="qt_pool", bufs=8))
        w_pool = ph_all.enter_context(tc.tile_pool(name="w_pool", bufs=8))
        qt_sb = [qt_pool.tile([P, L], BF16, tag="qt", name=f"qtsb{t}") for t in range(8)]
        wq_sb = [w_pool.tile([P, IL], BF16, tag="wq", name=f"wq{t}") for t in range(8)]
        wk_sb = [w_pool.tile([P, IL], BF16, tag="wk", name=f"wk{t}") for t in range(8)]
        wv_sb = [w_pool.tile([P, IL], BF16, tag="wv", name=f"wv{t}") for t in range(8)]
        for t in range(8):
            nc.sync.dma_start(out=qt_sb[t], in_=qt_d[t * P : (t + 1) * P, :])
            nc.sync.dma_start(out=wq_sb[t], in_=wq_d[t * P : (t + 1) * P, :])
            nc.sync.dma_start(out=wk_sb[t], in_=wk_d[t * P : (t + 1) * P, :])
        for t in range(8):
            nc.sync.dma_start(out=wv_sb[t], in_=wv_d[t * P : (t + 1) * P, :])

        # q/k projection Mtiles 0..1 up front (4-bank psum pool, then closed)
        with tc.tile_pool(name="qk_ps", bufs=2, space="PSUM") as qk_ps:
            def qk_mtile(m):
                for half in range(2):
                    for w_sb, bias_sb, dest, nm in (
                        (wq_sb, bq_sb, qT, "q"),
                        (wk_sb, bk_sb, kT, "k"),
                    ):
                        ps = qk_ps.tile(
                            [P, L // 2], F32, tag="qkps", name=f"ps{nm}{m}{half}"
                        )
                        for t in range(8):
                            for c in range(2):
                                nc.tensor.matmul(
                                    ps[:, c * 512 : (c + 1) * 512],
                                    w_sb[t][:, m * P : (m + 1) * P],
                                    qt_sb[t][
                                        :,
                                        (2 * half + c) * 512 : (2 * half + c + 1) * 512,
                                    ],
                                    start=(t == 0),
                                    stop=(t == 7),
                                )
                        nc.vector.tensor_scalar_add(
                            out=dest[m][:, half * 1024 : (half + 1) * 1024],
                            in0=ps,
                            scalar1=bias_sb[:, m : m + 1],
                        )

            qk_mtile(0)
            qk_mtile(1)

        # ---------------- phase 2: attention with interleaved fillers ----------
        with ExitStack() as ph2:
            at_pools = [
                ph2.enter_context(tc.tile_pool(name=f"at{i}", bufs=3)) for i in (0, 1)
            ]
            small = ph2.enter_context(tc.tile_pool(name="small", bufs=4))
            osb = ph2.enter_context(tc.tile_pool(name="osb", bufs=3))
            st_ps = [
                ph2.enter_context(tc.tile_pool(name=f"st{i}", bufs=1, space="PSUM"))
                for i in (0, 1)
            ]
            pv_ps = [
                ph2.enter_context(tc.tile_pool(name=f"pv{i}", bufs=1, space="PSUM"))
                for i in (0, 1)
            ]
            den_ps = ph2.enter_context(tc.tile_pool(name="den", bufs=1, space="PSUM"))

            fillers = []  # deque of thunks, each ~0.5-2us of PE work

            def make_qk23_fillers(ps_pool):
                for m in (2, 3):
                    for w_sb, bias_sb, dest, nm in (
                        (wq_sb, bq_sb, qT, "q"),
                        (wk_sb, bk_sb, kT, "k"),
                    ):
                        for ch in range(4):
                            def thunk(m=m, w_sb=w_sb, bias_sb=bias_sb, dest=dest,
                                      nm=nm, ch=ch):
                                ps = ps_pool.tile(
                                    [P, 512], F32, tag="qk2",
                                    name=f"p2{nm}{m}{ch}",
                                )
                                for t in range(8):
                                    nc.tensor.matmul(
                                        ps,
                                        w_sb[t][:, m * P : (m + 1) * P],
                                        qt_sb[t][:, ch * 512 : (ch + 1) * 512],
                                        start=(t == 0),
                                        stop=(t == 7),
                                    )
                                nc.vector.tensor_scalar_add(
                                    out=dest[m][:, ch * 512 : (ch + 1) * 512],
                                    in0=ps,
                                    scalar1=bias_sb[:, m : m + 1],
                                )
                            fillers.append(thunk)

            def make_outproj_fillers(ps_pool, lts):
                for lt in lts:
                    for c in (0, 1):
                        def thunk(lt=lt, c=c):
                            ps = ps_pool.tile(
                                [P, 512], F32, tag="ops", name=f"ops{lt}{c}"
                            )
                            for k in range(4):
                                nc.tensor.matmul(
                                    ps,
                                    aoT[k][:, lt * P : (lt + 1) * P],
                                    opt_sb[k][:, c * 512 : (c + 1) * 512],
                                    start=(k == 0),
                                    stop=(k == 3),
                                )
                            ob = osb.tile([P, 512], F32, tag="ob", name=f"ob{lt}{c}")
                            nc.vector.tensor_copy(out=ob, in_=ps)
                            nc.sync.dma_start(
                                out=out_d[lt * P : (lt + 1) * P, c * 512 : (c + 1) * 512],
                                in_=ob,
                            )
                        fillers.append(thunk)

            def attn_chunk(rnd, lq, v_interleave, fill_budget):
                lanes = (2 * rnd, 2 * rnd + 1)
                lqs = slice(lq * 512, (lq + 1) * 512)
                den_t = den_ps.tile([P, 512], F32, tag="den", name=f"den_{rnd}_{lq}")
                pv_t = {}
                for i, p in enumerate(lanes):
                    pv_t[p] = pv_ps[i].tile(
                        [P, 512], F32, tag="pv", name=f"pv_{p}_{lq}"
                    )

                def pv_den_step(lk, ats):
                    for i, p in enumerate(lanes):
                        for j in (0, 1):
                            nc.tensor.matmul(
                                pv_t[p][64 * j : 64 * j + 64, :],
                                vv[lk][:, P * p + 64 * j : P * p + 64 * j + 64],
                                ats[i][:, j, :],
                                start=(lk == 0),
                                stop=(lk == 15),
                            )
                    for i, p in enumerate(lanes):
                        for j in (0, 1):
                            r0 = 64 * i + 32 * j
                            nc.tensor.matmul(
                                den_t[r0 : r0 + 1, :],
                                ones_col,
                                ats[i][:, j, :],
                                start=(lk == 0),
                                stop=(lk == 15),
                                tile_position=(0, r0),
                            )

                prev = None
                for lk in range(16):
                    lks = slice(lk * P, (lk + 1) * P)
                    ats = []
                    for i, p in enumerate(lanes):
                        st = st_ps[i].tile(
                            [P, 2, 512], F32, tag="st", name=f"st_{p}_{lq}_{lk}"
                        )
                        for j in (0, 1):
                            nc.tensor.matmul(
                                st[:, j, :],
                                kT[p][64 * j : 64 * j + 64, lks],
                                qT[p][64 * j : 64 * j + 64, lqs],
                                start=True,
                                stop=True,
                            )
                        at = at_pools[i].tile(
                            [P, 2, 512], BF16, tag="at", name=f"at_{p}_{lq}_{lk}"
                        )
                        nc.scalar.activation(out=at, in_=st, func=EXP)
                        ats.append(at)
                    if v_interleave is not None:
                        v_interleave(lk)
                    for _ in range(fill_budget):
                        if fillers:
                            fillers.pop(0)()
                    if prev is not None:
                        pv_den_step(lk - 1, prev)
                    prev = ats
                pv_den_step(15, prev)

                for i, p in enumerate(lanes):
                    bcs = small.tile(
                        [P, 2, 512], F32, tag="bcs", name=f"bcs_{p}_{lq}", bufs=2
                    )
                    rc = small.tile(
                        [1, 2, 512], F32, tag="rc", name=f"rc_{p}_{lq}", bufs=2
                    )
                    for j in (0, 1):
                        r0 = 64 * i + 32 * j
                        nc.vector.reciprocal(out=rc[:, j, :], in_=den_t[r0 : r0 + 1, :])
                    nc.gpsimd.partition_broadcast(bcs, rc)
                    for j in (0, 1):
                        nc.vector.tensor_mul(
                            out=aoT[p][64 * j : 64 * j + 64, lqs],
                            in0=pv_t[p][64 * j : 64 * j + 64, :],
                            in1=bcs[64 * j : 64 * j + 64, j, :],
                        )

            # round 0, chunk 0: v projection rides inside the lk loop
            with tc.tile_pool(name="v_ps", bufs=1, space="PSUM") as v_ps:
                def v_interleave(lk):
                    ps = v_ps.tile([P, IL], F32, tag="vps", name=f"psv{lk}")
                    for t in range(8):
                        nc.tensor.matmul(
                            ps,
                            qt_sb[t][:, lk * P : (lk + 1) * P],
                            wv_sb[t],
                            start=(t == 0),
                            stop=(t == 7),
                        )
                    nc.vector.tensor_add(out=vv[lk], in0=ps, in1=bvb_sb)

                attn_chunk(0, 0, v_interleave, 0)

            # round 0, chunks 1-3: q/k Mtiles 2,3 fill PE idle
            with tc.tile_pool(name="qk2_ps", bufs=1, space="PSUM") as qk2_ps:
                make_qk23_fillers(qk2_ps)
                for lq in range(1, 4):
                    attn_chunk(0, lq, None, 1)
                while fillers:
                    fillers.pop(0)()

            # round 1: out-projection of previous chunks fills PE idle
            with tc.tile_pool(name="o_ps", bufs=1, space="PSUM") as o_ps:
                for lq in range(4):
                    if lq >= 1:
                        make_outproj_fillers(o_ps, range(4 * (lq - 1), 4 * lq))
                    attn_chunk(1, lq, None, 1)
                make_outproj_fillers(o_ps, range(12, 16))
                while fillers:
                    fillers.pop(0)()

        est.close()

      if lctx is not None:
          lctx.__exit__(None, None, None)

    nc.compile()
    return nc


def _prep_inputs(query, qkv_proj, qkv_bias, out_proj):
    """Per-core input shards (host-side)."""
    query = np.asarray(query, dtype=np.float32)
    qkv_proj = np.asarray(qkv_proj, dtype=np.float32)
    qkv_bias = np.asarray(qkv_bias, dtype=np.float32)
    W3 = qkv_proj.reshape(E, 3, E)  # [i, c, e], row f = 3*i + c
    b3 = qkv_bias.reshape(E, 3)
    bf = ml_dtypes.bfloat16
    maps = []
    for c in range(8):
        n, half = c // 2, c % 2
        isl = slice(IL * half, IL * half + IL)
        maps.append(
            {
                "qt": np.ascontiguousarray(query[:, n, :].T).astype(bf),
                "wq": np.ascontiguousarray(W3[isl, 0, :].T * SCALE).astype(bf),
                "wk": np.ascontiguousarray(W3[isl, 1, :].T).astype(bf),
                "wv": np.ascontiguousarray(W3[isl, 2, :].T).astype(bf),
                "bq": np.ascontiguousarray((b3[isl, 0] * SCALE).reshape(4, P)),
                "bk": np.ascontiguousarray(b3[isl, 1].reshape(4, P)),
                "bvb": np.ascontiguousarray(np.broadcast_to(b3[isl, 2], (P, IL))),
                "opt": np.ascontiguousarray(out_proj[:, isl].T).astype(bf),
            }
        )
    return maps


def kernel(query, qkv_proj, qkv_bias, out_proj, out_bias, **run_kwargs):
    global _built
    out_proj = np.asarray(out_proj, dtype=np.float32)
    out_bias = np.asarray(out_bias, dtype=np.float32)
    if _built is None:
        _built = build()
    in_maps = _prep_inputs(query, qkv_proj, qkv_bias, out_proj)
    res = run_bass_kernel_spmd(_built, in_maps, core_ids=list(range(8)), **run_kwargs)
    parts = [r["out"] for r in res.results]
    out = np.empty((L, N, E), dtype=np.float32)
    for n in range(N):
        out[:, n, :] = parts[2 * n] + parts[2 * n + 1] + out_bias
    kernel.last_result = res
    return out



# revision 31
# speedup vs baseline: 1.4501x; 1.0393x over previous
"""Multi-head self-attention (L=2048, N=4, E=1024, h=16) on 8 NeuronCores.

Sharding: core c handles batch n = c//2 and heads [8*(c%2), 8*(c%2)+8).
Each core computes q/k/v projections for its (n, head-block), attention,
and a partial out-projection (columns of out_proj for its heads).
Host sums the two partials per batch n and adds out_bias.

PE strategy (all operands bf16, accumulation fp32 in PSUM):
- q/k/v projections: K=128 matmuls over 8 E-tiles.
- QK^T: row-packed pairs (two K=64 matmuls on row groups 0-1/2-3 run
  concurrently in the PE array).
- softmax: no max-subtraction (scores are small by construction);
  denominators via M=1 ones-matmuls, 4 heads col-packed per 32-strips;
  reciprocal on DVE, broadcast via gpsimd partition_broadcast.
- attn @ V: col-packed pairs (M=64 at tile_position (0,0)/(0,64)).
- out projection: K=128 over 4 stacked head-pair tiles.
"""

from contextlib import ExitStack

import ml_dtypes
import numpy as np

import concourse.bacc as bacc
import concourse.mybir as mybir
import concourse.tile as tile
from concourse.bass_utils import run_bass_kernel_spmd

L, N, E, H, D = 2048, 4, 1024, 16, 64
SCALE = D**-0.5
IL = 512  # inner dims per core (8 heads * 64)
P = 128
F32 = mybir.dt.float32
BF16 = mybir.dt.bfloat16
I16 = mybir.dt.int16
EXP = mybir.ActivationFunctionType.Exp

# Schraudolph exp on the DVE: int16 bits = round(s*128*log2(e) + C2) ARE the
# bf16 value of ~exp(s) (max rel err ~3%). Used for a fraction of the score
# tiles so the PE's next QK matmul (blocked on the score-PSUM WAR) doesn't
# always wait for ScalarE.
SCH_C1 = 184.6650  # 128*log2(e)
SCH_C2 = 16250.49  # 127*128 - 5.51 (centering); HW fp32->int16 rounds

_built = None


def build(dbg=False, reps=1, loop_reps=1):
    nc = bacc.Bacc("TRN2", target_bir_lowering=False, debug=False, num_devices=8)

    qt_d = nc.dram_tensor("qt", [E, L], BF16, kind="ExternalInput")
    wq_d = nc.dram_tensor("wq", [E, IL], BF16, kind="ExternalInput")
    wk_d = nc.dram_tensor("wk", [E, IL], BF16, kind="ExternalInput")
    wv_d = nc.dram_tensor("wv", [E, IL], BF16, kind="ExternalInput")
    bq_d = nc.dram_tensor("bq", [4, P], F32, kind="ExternalInput")
    bk_d = nc.dram_tensor("bk", [4, P], F32, kind="ExternalInput")
    bvb_d = nc.dram_tensor("bvb", [P, IL], F32, kind="ExternalInput")
    opt_d = nc.dram_tensor("opt", [IL, E], BF16, kind="ExternalInput")
    out_d = nc.dram_tensor("out", [L, E], F32, kind="ExternalOutput")

    with tile.TileContext(nc) as tc:
      lctx = tc.For_i(0, loop_reps, 1) if loop_reps > 1 else None
      if lctx is not None:
          lctx.__enter__()
      for _rep in range(reps):
        est = ExitStack()
        persist = est.enter_context(tc.tile_pool(name="persist", bufs=1))

        ones_col = persist.tile([P, 1], BF16, name="ones_col")
        nc.vector.memset(ones_col, 1.0)

        bq_sb = persist.tile([P, 4], F32, name="bq_sb")
        bk_sb = persist.tile([P, 4], F32, name="bk_sb")
        for m in range(4):
            nc.sync.dma_start(out=bq_sb[:, m : m + 1], in_=bq_d[m, :, None])
            nc.sync.dma_start(out=bk_sb[:, m : m + 1], in_=bk_d[m, :, None])
        bvb_sb = persist.tile([P, IL], F32, name="bvb_sb")
        nc.sync.dma_start(out=bvb_sb, in_=bvb_d[:, :])

        qT = [persist.tile([P, L], BF16, name=f"qT{m}") for m in range(4)]
        kT = [persist.tile([P, L], BF16, name=f"kT{m}") for m in range(4)]
        vv = [persist.tile([P, IL], BF16, name=f"v{t}") for t in range(16)]
        aoT = [persist.tile([P, L], BF16, name=f"aoT{m}") for m in range(4)]
        opt_sb = [persist.tile([P, E], BF16, name=f"opt{k}") for k in range(4)]
        for k in range(4):
            nc.sync.dma_start(out=opt_sb[k], in_=opt_d[k * P : (k + 1) * P, :])

        # ---------------- phase 1 setup: streaming inputs ----------------
        ph_all = est.enter_context(ExitStack())
        qt_pool = ph_all.enter_context(tc.tile_pool(name="qt_pool", bufs=8))
        w_pool = ph_all.enter_context(tc.tile_pool(name="w_pool", bufs=8))
        qt_sb = [qt_pool.tile([P, L], BF16, tag="qt", name=f"qtsb{t}") for t in range(8)]
        wq_sb = [w_pool.tile([P, IL], BF16, tag="wq", name=f"wq{t}") for t in range(8)]
        wk_sb = [w_pool.tile([P, IL], BF16, tag="wk", name=f"wk{t}") for t in range(8)]
        wv_sb = [w_pool.tile([P, IL], BF16, tag="wv", name=f"wv{t}") for t in range(8)]
        for t in range(8):
            nc.sync.dma_start(out=qt_sb[t], in_=qt_d[t * P : (t + 1) * P, :])
            nc.sync.dma_start(out=wq_sb[t], in_=wq_d[t * P : (t + 1) * P, :])
            nc.sync.dma_start(out=wk_sb[t], in_=wk_d[t * P : (t + 1) * P, :])
        for t in range(8):
            nc.sync.dma_start(out=wv_sb[t], in_=wv_d[t * P : (t + 1) * P, :])

        # q/k projection Mtiles 0..1 up front (4-bank psum pool, then closed)
        with tc.tile_pool(name="qk_ps", bufs=2, space="PSUM") as qk_ps:
            def qk_mtile(m):
                for half in range(2):
                    for w_sb, bias_sb, dest, nm in (
                        (wq_sb, bq_sb, qT, "q"),
                        (wk_sb, bk_sb, kT, "k"),
                    ):
                        ps = qk_ps.tile(
                            [P, L // 2], F32, tag="qkps", name=f"ps{nm}{m}{half}"
                        )
                        for t in range(8):
                            for c in range(2):
                                nc.tensor.matmul(
                                    ps[:, c * 512 : (c + 1) * 512],
                                    w_sb[t][:, m * P : (m + 1) * P],
                                    qt_sb[t][
                                        :,
                                        (2 * half + c) * 512 : (2 * half + c + 1) * 512,
                                    ],
                                    start=(t == 0),
                                    stop=(t == 7),
                                )
                        nc.vector.tensor_scalar_add(
                            out=dest[m][:, half * 1024 : (half + 1) * 1024],
                            in0=ps,
                            scalar1=bias_sb[:, m : m + 1],
                        )

            qk_mtile(0)
            qk_mtile(1)

        # ---------------- phase 2: attention with interleaved fillers ----------
        with ExitStack() as ph2:
            at_pools = [
                ph2.enter_context(tc.tile_pool(name=f"at{i}", bufs=3)) for i in (0, 1)
            ]
            small = ph2.enter_context(tc.tile_pool(name="small", bufs=4))
            osb = ph2.enter_context(tc.tile_pool(name="osb", bufs=3))
            st_ps = [
                ph2.enter_context(tc.tile_pool(name=f"st{i}", bufs=1, space="PSUM"))
                for i in (0, 1)
            ]
            pv_ps = [
                ph2.enter_context(tc.tile_pool(name=f"pv{i}", bufs=1, space="PSUM"))
                for i in (0, 1)
            ]
            den_ps = ph2.enter_context(tc.tile_pool(name="den", bufs=1, space="PSUM"))

            fillers = []  # deque of thunks, each ~0.5-2us of PE work

            def make_qk23_fillers(ps_pool):
                for m in (2, 3):
                    for w_sb, bias_sb, dest, nm in (
                        (wq_sb, bq_sb, qT, "q"),
                        (wk_sb, bk_sb, kT, "k"),
                    ):
                        for ch in range(4):
                            def thunk(m=m, w_sb=w_sb, bias_sb=bias_sb, dest=dest,
                                      nm=nm, ch=ch):
                                ps = ps_pool.tile(
                                    [P, 512], F32, tag="qk2",
                                    name=f"p2{nm}{m}{ch}",
                                )
                                for t in range(8):
                                    nc.tensor.matmul(
                                        ps,
                                        w_sb[t][:, m * P : (m + 1) * P],
                                        qt_sb[t][:, ch * 512 : (ch + 1) * 512],
                                        start=(t == 0),
                                        stop=(t == 7),
                                    )
                                nc.vector.tensor_scalar_add(
                                    out=dest[m][:, ch * 512 : (ch + 1) * 512],
                                    in0=ps,
                                    scalar1=bias_sb[:, m : m + 1],
                                )
                            fillers.append(thunk)

            def make_outproj_fillers(ps_pool, lts):
                for lt in lts:
                    for c in (0, 1):
                        def thunk(lt=lt, c=c):
                            ps = ps_pool.tile(
                                [P, 512], F32, tag="ops", name=f"ops{lt}{c}"
                            )
                            for k in range(4):
                                nc.tensor.matmul(
                                    ps,
                                    aoT[k][:, lt * P : (lt + 1) * P],
                                    opt_sb[k][:, c * 512 : (c + 1) * 512],
                                    start=(k == 0),
                                    stop=(k == 3),
                                )
                            ob = osb.tile([P, 512], F32, tag="ob", name=f"ob{lt}{c}")
                            nc.vector.tensor_copy(out=ob, in_=ps)
                            nc.sync.dma_start(
                                out=out_d[lt * P : (lt + 1) * P, c * 512 : (c + 1) * 512],
                                in_=ob,
                            )
                        fillers.append(thunk)

            def attn_chunk(rnd, lq, v_interleave, fill_budget):
                lanes = (2 * rnd, 2 * rnd + 1)
                lqs = slice(lq * 512, (lq + 1) * 512)
                den_t = den_ps.tile([P, 512], F32, tag="den", name=f"den_{rnd}_{lq}")
                pv_t = {}
                for i, p in enumerate(lanes):
                    pv_t[p] = pv_ps[i].tile(
                        [P, 512], F32, tag="pv", name=f"pv_{p}_{lq}"
                    )

                def pv_den_step(lk, ats):
                    for i, p in enumerate(lanes):
                        for j in (0, 1):
                            nc.tensor.matmul(
                                pv_t[p][64 * j : 64 * j + 64, :],
                                vv[lk][:, P * p + 64 * j : P * p + 64 * j + 64],
                                ats[i][:, j, :],
                                start=(lk == 0),
                                stop=(lk == 15),
                            )
                    for i, p in enumerate(lanes):
                        for j in (0, 1):
                            r0 = 64 * i + 32 * j
                            nc.tensor.matmul(
                                den_t[r0 : r0 + 1, :],
                                ones_col,
                                ats[i][:, j, :],
                                start=(lk == 0),
                                stop=(lk == 15),
                                tile_position=(0, r0),
                            )

                prev = None
                for lk in range(16):
                    lks = slice(lk * P, (lk + 1) * P)
                    ats = []
                    for i, p in enumerate(lanes):
                        st = st_ps[i].tile(
                            [P, 2, 512], F32, tag="st", name=f"st_{p}_{lq}_{lk}"
                        )
                        for j in (0, 1):
                            nc.tensor.matmul(
                                st[:, j, :],
                                kT[p][64 * j : 64 * j + 64, lks],
                                qT[p][64 * j : 64 * j + 64, lqs],
                                start=True,
                                stop=True,
                            )
                        at = at_pools[i].tile(
                            [P, 2, 512], BF16, tag="at", name=f"at_{p}_{lq}_{lk}"
                        )
                        if i == 1 and lk % 2 == 0 and 2 <= lk <= 13:
                            nc.vector.tensor_scalar(
                                out=at.bitcast(I16),
                                in0=st,
                                scalar1=SCH_C1,
                                scalar2=SCH_C2,
                                op0=mybir.AluOpType.mult,
                                op1=mybir.AluOpType.add,
                            )
                        else:
                            nc.scalar.activation(out=at, in_=st, func=EXP)
                        ats.append(at)
                    if v_interleave is not None:
                        v_interleave(lk)
                    for _ in range(fill_budget):
                        if fillers:
                            fillers.pop(0)()
                    if prev is not None:
                        pv_den_step(lk - 1, prev)
                    prev = ats
                pv_den_step(15, prev)

                for i, p in enumerate(lanes):
                    bcs = small.tile(
                        [P, 2, 512], F32, tag="bcs", name=f"bcs_{p}_{lq}", bufs=2
                    )
                    rc = small.tile(
                        [1, 2, 512], F32, tag="rc", name=f"rc_{p}_{lq}", bufs=2
                    )
                    for j in (0, 1):
                        r0 = 64 * i + 32 * j
                        nc.vector.reciprocal(out=rc[:, j, :], in_=den_t[r0 : r0 + 1, :])
                    nc.gpsimd.partition_broadcast(bcs, rc)
                    for j in (0, 1):
                        nc.vector.tensor_mul(
                            out=aoT[p][64 * j : 64 * j + 64, lqs],
                            in0=pv_t[p][64 * j : 64 * j + 64, :],
                            in1=bcs[64 * j : 64 * j + 64, j, :],
                        )

            # round 0, chunk 0: v projection rides inside the lk loop
            with tc.tile_pool(name="v_ps", bufs=1, space="PSUM") as v_ps:
                def v_interleave(lk):
                    ps = v_ps.tile([P, IL], F32, tag="vps", name=f"psv{lk}")
                    for t in range(8):
                        nc.tensor.matmul(
                            ps,
                            qt_sb[t][:, lk * P : (lk + 1) * P],
                            wv_sb[t],
                            start=(t == 0),
                            stop=(t == 7),
                        )
                    nc.vector.tensor_add(out=vv[lk], in0=ps, in1=bvb_sb)

                attn_chunk(0, 0, v_interleave, 0)

            # round 0, chunks 1-3: q/k Mtiles 2,3 fill PE idle
            with tc.tile_pool(name="qk2_ps", bufs=1, space="PSUM") as qk2_ps:
                make_qk23_fillers(qk2_ps)
                for lq in range(1, 4):
                    attn_chunk(0, lq, None, 1)
                while fillers:
                    fillers.pop(0)()

            # round 1: out-projection of previous chunks fills PE idle
            with tc.tile_pool(name="o_ps", bufs=1, space="PSUM") as o_ps:
                for lq in range(4):
                    if lq >= 1:
                        make_outproj_fillers(o_ps, range(4 * (lq - 1), 4 * lq))
                    attn_chunk(1, lq, None, 1)
                make_outproj_fillers(o_ps, range(12, 16))
                while fillers:
                    fillers.pop(0)()

        est.close()

      if lctx is not None:
          lctx.__exit__(None, None, None)

    nc.compile()
    return nc


def _prep_inputs(query, qkv_proj, qkv_bias, out_proj):
    """Per-core input shards (host-side)."""
    query = np.asarray(query, dtype=np.float32)
    qkv_proj = np.asarray(qkv_proj, dtype=np.float32)
    qkv_bias = np.asarray(qkv_bias, dtype=np.float32)
    W3 = qkv_proj.reshape(E, 3, E)  # [i, c, e], row f = 3*i + c
    b3 = qkv_bias.reshape(E, 3)
    bf = ml_dtypes.bfloat16
    maps = []
    for c in range(8):
        n, half = c // 2, c % 2
        isl = slice(IL * half, IL * half + IL)
        maps.append(
            {
                "qt": np.ascontiguousarray(query[:, n, :].T).astype(bf),
                "wq": np.ascontiguousarray(W3[isl, 0, :].T * SCALE).astype(bf),
                "wk": np.ascontiguousarray(W3[isl, 1, :].T).astype(bf),
                "wv": np.ascontiguousarray(W3[isl, 2, :].T).astype(bf),
                "bq": np.ascontiguousarray((b3[isl, 0] * SCALE).reshape(4, P)),
                "bk": np.ascontiguousarray(b3[isl, 1].reshape(4, P)),
                "bvb": np.ascontiguousarray(np.broadcast_to(b3[isl, 2], (P, IL))),
                "opt": np.ascontiguousarray(out_proj[:, isl].T).astype(bf),
            }
        )
    return maps


def kernel(query, qkv_proj, qkv_bias, out_proj, out_bias, **run_kwargs):
    global _built
    out_proj = np.asarray(out_proj, dtype=np.float32)
    out_bias = np.asarray(out_bias, dtype=np.float32)
    if _built is None:
        _built = build()
    in_maps = _prep_inputs(query, qkv_proj, qkv_bias, out_proj)
    res = run_bass_kernel_spmd(_built, in_maps, core_ids=list(range(8)), **run_kwargs)
    parts = [r["out"] for r in res.results]
    out = np.empty((L, N, E), dtype=np.float32)
    for n in range(N):
        out[:, n, :] = parts[2 * n] + parts[2 * n + 1] + out_bias
    kernel.last_result = res
    return out

